# revision 1
# baseline (speedup 1.0000x reference)
"""Multi-head attention Trainium2 Bass kernel (8 NeuronCores).

Problem: B=2, S=2048, D=1024, H=16, Dh=64, scale=1/sqrt(D).
Sharding: batch x head. Core c handles batch c//4, heads (c%4)*4 .. +4.
No collectives: per-core partial outputs are combined on the host
(softmax normalization + head sum + b_o add), which is O(B*H*S*Dh) work.

Per-core pipeline (all matmuls via TensorE, fp32 storage / float32r
matmul dtype, attention probabilities in bf16):
  1. QKV projection from pre-transposed xT [D,S]:
       qT,kT per head-pair in [128, S] tiles (q rows 0-63 / 64-127 ...),
       v in natural [sk, d] layout with a fused ones column per head.
  2. Attention per (q-block of 512, head):
       logitsT [sk,sq] = k_chunk @ qT  (16 chunks of 128 sk)
       exp on ScalarE (PSUM -> bf16 SBUF), multiplicative {0,1} bf16
       mask on VectorE (2x mode), then ctxT'[65, 512] accumulated as
       (v|1).T @ P — row 64 = softmax denominators.
  3. Per-head output projection Wo_h.T @ ctxuT on device; DMA psum->HBM.
"""

import numpy as np
import ml_dtypes

import concourse.bass as bass  # noqa: F401
import concourse.tile as tile
from concourse import bacc, mybir
from concourse.bass_utils import run_bass_kernel_spmd

B, S, D = 2, 2048, 1024
H, Dh = 16, 64
NCORE = 8
GPB = NCORE // B            # cores per batch (4)
HL = H // GPB               # local heads per core (4)
SCALE = float(1.0 / np.sqrt(np.float32(D)))

F32 = mybir.dt.float32
F32R = mybir.dt.float32r
BF16 = mybir.dt.bfloat16

# dtype config (tweakable for precision/perf experiments)
MM_CAST = F32R   # dtype used for fp32-stored matmul operands
PT_DT = BF16     # attention probability storage
V_DT = BF16      # v storage (must match PT_DT for the ctx matmul)

ABLATE = set()   # subset of {"exp","mask","logits","ctx","qkv","proj","dma_mask","dma_x"}

ND = D // 128    # 8 contraction chunks
NCH = S // 128   # 16 sk chunks
NQG = S // 512   # 4 query groups


def build_module(reps=1, ablate=()):
    ablate = set(ablate)
    nc = bacc.Bacc("TRN2", target_bir_lowering=False, debug=False,
                   num_devices=NCORE)

    xT = nc.dram_tensor("xT", [D, S], F32, kind="ExternalInput").ap()
    wqk = nc.dram_tensor("wqk", [D, 2 * HL * Dh], F32, kind="ExternalInput").ap()
    wv = nc.dram_tensor("wv", [D, HL * Dh], F32, kind="ExternalInput").ap()
    bqk = nc.dram_tensor("bqk", [128, 4], F32, kind="ExternalInput").ap()
    bv = nc.dram_tensor("bv", [128, HL * Dh], F32, kind="ExternalInput").ap()
    wo = nc.dram_tensor("wo", [Dh, HL * Dh], F32, kind="ExternalInput").ap()
    maskT = nc.dram_tensor("maskT", [S, S], BF16, kind="ExternalInput").ap()
    outp = nc.dram_tensor("outp", [reps * HL, Dh, S], F32,
                      kind="ExternalOutput").ap()
    ssum = nc.dram_tensor("ssum", [reps * HL, S], F32,
                      kind="ExternalOutput").ap()

    with tile.TileContext(nc) as tc:
        if True:
            # ---------------- persistent tiles ----------------
            with (
                tc.tile_pool(name="const", bufs=1) as constp,
                tc.tile_pool(name="qk", bufs=1) as qkp,
                tc.tile_pool(name="vpool", bufs=1) as vpoolp,
            ):
                wqk_sb = constp.tile([128, ND * 512], F32R)
                wv_sb = constp.tile([128, ND * 256], F32R)
                wo_sb = constp.tile([Dh, HL * Dh], F32R)
                bqk_sb = constp.tile([128, 4], F32)
                nc.sync.dma_start(bqk_sb, bqk)
                bv_sb = constp.tile([128, HL * Dh], F32)
                nc.sync.dma_start(bv_sb, bv)
                with tc.tile_pool(name="wstage", bufs=1, side="right") as wstp:
                    wqk_st = wstp.tile([128, ND * 512], F32)
                    nc.sync.dma_start(
                        wqk_st.rearrange("p (d c) -> p d c", d=ND),
                        wqk.rearrange("(d p) c -> p d c", p=128))
                    nc.vector.tensor_copy(wqk_sb, wqk_st)
                    wv_st = wstp.tile([128, ND * 256], F32)
                    nc.sync.dma_start(
                        wv_st.rearrange("p (d c) -> p d c", d=ND),
                        wv.rearrange("(d p) c -> p d c", p=128))
                    nc.vector.tensor_copy(wv_sb, wv_st)
                    wo_st = wstp.tile([Dh, HL * Dh], F32)
                    nc.sync.dma_start(wo_st, wo)
                    nc.vector.tensor_copy(wo_sb, wo_st)

                # qT / kT per head pair: rows 0-63 head 2g, rows 64-127 head 2g+1
                qt = [qkp.tile([128, S], F32R, name=f"qt{g}", tag=f"qt{g}")
                      for g in range(HL // 2)]
                kt = [qkp.tile([128, S], F32R, name=f"kt{g}", tag=f"kt{g}")
                      for g in range(HL // 2)]
                # v in [sk, d] layout: per sk-chunk j, per head h: 64 v cols + ones
                v_sb = vpoolp.tile([128, NCH * HL * 65], V_DT)
                nc.vector.memset(
                    v_sb.rearrange("p (m c) -> p m c", c=65)[:, :, 64:65], 1.0)

            # ---------------- repeated compute (reps>1 for timing) ----
            for _rep in range(reps):
                # ---------------- phase 1: QKV projection ----------------
                with (
                    tc.tile_pool(name="xtp", bufs=2, side="right") as xtp,
                    tc.tile_pool(name="pqk", space="PSUM", bufs=4) as pqkp,
                    tc.tile_pool(name="pv", space="PSUM", bufs=4) as pvp,
                ):
                    for sb4 in range(NQG):
                        xt_st = xtp.tile([128, ND * 512], F32, tag="xts", bufs=1)
                        if "dma_x" not in ablate:
                            nc.sync.dma_start(
                                xt_st.rearrange("p (d c) -> p d c", d=ND),
                                xT.rearrange("(d p) s -> p d s", p=128)
                                  [:, :, sb4 * 512:(sb4 + 1) * 512])
                        xt = xtp.tile([128, ND * 512], F32R, tag="xt", bufs=1)
                        nc.vector.tensor_copy(xt, xt_st)
                        # q/k pair blocks: 0,1 = q pairs; 2,3 = k pairs
                        for blk in range(4):
                            ps = pqkp.tile([128, 512], F32, tag="pqk", bufs=4)
                            for d in range(ND if "qkv" not in ablate else 0):
                                nc.tensor.matmul(
                                    ps,
                                    lhsT=wqk_sb[:, d * 512 + blk * 128:
                                                d * 512 + (blk + 1) * 128],
                                    rhs=xt[:, d * 512:(d + 1) * 512],
                                    start=(d == 0), stop=(d == ND - 1))
                            tgt = qt[blk] if blk < 2 else kt[blk - 2]
                            nc.vector.tensor_scalar_add(
                                tgt[:, sb4 * 512:(sb4 + 1) * 512], ps,
                                bqk_sb[:, blk:blk + 1])
                        # v chunks j = 4*sb4 .. +4
                        for jj in range(4):
                            j = sb4 * 4 + jj
                            psv = pvp.tile([128, HL * Dh], F32, tag="pv", bufs=4)
                            for d in range(ND if "qkv" not in ablate else 0):
                                nc.tensor.matmul(
                                    psv,
                                    lhsT=xt[:, d * 512 + jj * 128:
                                            d * 512 + jj * 128 + 128],
                                    rhs=wv_sb[:, d * 256:(d + 1) * 256],
                                    start=(d == 0), stop=(d == ND - 1))
                            nc.vector.tensor_add(
                                v_sb[:, j * (HL * 65):(j + 1) * (HL * 65)]
                                    .rearrange("p (h c) -> p h c", h=HL)[:, :, 0:64],
                                psv.rearrange("p (h c) -> p h c", h=HL),
                                bv_sb.rearrange("p (h c) -> p h c", h=HL))

                # ---------------- phase 2: attention ----------------
                # Heads are processed in pairs (2g, 2g+1): their logits matmuls
                # use disjoint PE row groups (partitions 0-63 / 64-127) and run
                # concurrently. PT layout per round: [128, (j, head, 512)].
                with (
                    tc.tile_pool(name="maskp", bufs=3, side="right") as maskp,
                    tc.tile_pool(name="ptp", bufs=3, side="right") as ptp,
                    tc.tile_pool(name="ptps", space="PSUM", bufs=3) as ptpsp,
                    tc.tile_pool(name="ctxps", space="PSUM", bufs=2) as ctxpsp,
                    tc.tile_pool(name="cup", bufs=3, side="right") as cup,
                ):
                    for qg in range(NQG):
                        mts = []
                        for r in range(2):
                            mt = maskp.tile([128, 8 * 512], BF16, name=f"mt{r}",
                                            tag="mask", bufs=2)
                            if "dma_mask" not in ablate:
                                nc.sync.dma_start(
                                    mt.rearrange("p (j c) -> p j c", j=8),
                                    maskT.rearrange("(j p) q -> p j q", p=128)
                                         [:, 8 * r:8 * r + 8,
                                          qg * 512:(qg + 1) * 512])
                            mts.append(mt)
                        for g in range(HL // 2):
                            pts = []
                            for r in range(2):
                                ptt = ptp.tile([128, 8 * 2 * 512], PT_DT,
                                               name=f"pt{r}", tag="pt", bufs=2)
                                for jj in range(8):
                                    j = r * 8 + jj
                                    pps = ptpsp.tile([128, 1024], F32, tag="ptps",
                                                     bufs=3)
                                    if "logits" not in ablate:
                                        for hh in range(2):
                                            nc.tensor.matmul(
                                                pps[:, hh * 512:(hh + 1) * 512],
                                                lhsT=kt[g][hh * 64:(hh + 1) * 64,
                                                           j * 128:(j + 1) * 128],
                                                rhs=qt[g][hh * 64:(hh + 1) * 64,
                                                          qg * 512:(qg + 1) * 512],
                                                start=True, stop=True)
                                    if "exp" not in ablate:
                                        nc.scalar.activation(
                                            ptt[:, jj * 1024:(jj + 1) * 1024], pps,
                                            mybir.ActivationFunctionType.Exp)
                                if "mask" not in ablate:
                                    ptv = ptt.rearrange("p (j e c) -> p j e c",
                                                        j=8, e=2)
                                    mtv = mts[r].rearrange("p (j c) -> p j c", j=8)
                                    for e in range(2):
                                        nc.vector.tensor_mul(
                                            ptv[:, :, e, :], ptv[:, :, e, :], mtv)
                                pts.append(ptt)
                            for hh in range(2):
                                h = 2 * g + hh
                                ctx = ctxpsp.tile([65, 512], F32, tag="ctx", bufs=2)
                                for j in range(NCH if "ctx" not in ablate else 0):
                                    nc.tensor.matmul(
                                        ctx,
                                        lhsT=v_sb[:, j * (HL * 65) + h * 65:
                                                  j * (HL * 65) + (h + 1) * 65],
                                        rhs=pts[j // 8][:, (j % 8) * 1024
                                                        + hh * 512:
                                                        (j % 8) * 1024
                                                        + (hh + 1) * 512],
                                        start=(j == 0), stop=(j == NCH - 1))
                                cu = cup.tile([65, 512], F32R, tag="cu", bufs=3)
                                nc.vector.tensor_copy(cu, ctx)
                                nc.sync.dma_start(
                                    ssum[_rep * HL + h:_rep * HL + h + 1,
                                         qg * 512:(qg + 1) * 512],
                                    cu[64:65, :].bitcast(F32))
                                po = ptpsp.tile([64, 512], F32, tag="ptps")
                                if "proj" not in ablate:
                                    nc.tensor.matmul(
                                        po,
                                        lhsT=wo_sb[:, h * 64:(h + 1) * 64],
                                        rhs=cu[0:64, :],
                                        start=True, stop=True)
                                po_sb = cup.tile([64, 512], F32, tag="po", bufs=3)
                                nc.vector.tensor_copy(po_sb, po)
                                nc.sync.dma_start(
                                    outp[_rep * HL + h][:, qg * 512:(qg + 1) * 512],
                                    po_sb)

    nc.compile()
    return nc


_NC_CACHE = {}


def get_module(reps=1):
    if reps not in _NC_CACHE:
        _NC_CACHE[reps] = build_module(reps)
    return _NC_CACHE[reps]


def make_in_maps(x, W_qkv, b_qkv, W_o, b_o, mask):
    x = np.asarray(x, np.float32)
    W_qkv = np.asarray(W_qkv, np.float32)
    b_qkv = np.asarray(b_qkv, np.float32)
    W_o = np.asarray(W_o, np.float32)
    mask = np.asarray(mask)

    # reference layout: W_qkv[:, h*3*Dh + {0..Dh | Dh..2Dh | 2Dh..3Dh}] =
    # q|k|v of head h (qkv.reshape(B,S,H,3*Dh) then split on last axis)
    W3 = W_qkv.reshape(D, H, 3 * Dh)
    b3 = b_qkv.reshape(H, 3 * Dh)
    Wq = np.ascontiguousarray(W3[:, :, :Dh].reshape(D, H * Dh))
    Wk = np.ascontiguousarray(W3[:, :, Dh:2 * Dh].reshape(D, H * Dh))
    Wv = np.ascontiguousarray(W3[:, :, 2 * Dh:].reshape(D, H * Dh))
    bq = np.ascontiguousarray(b3[:, :Dh].reshape(H * Dh))
    bk = np.ascontiguousarray(b3[:, Dh:2 * Dh].reshape(H * Dh))
    bv_full = np.ascontiguousarray(b3[:, 2 * Dh:].reshape(H * Dh))

    xT_b = [np.ascontiguousarray(x[b].T) for b in range(B)]
    maskT_b = [np.ascontiguousarray(
        (mask[b, 0] != 0).T.astype(ml_dtypes.bfloat16)) for b in range(B)]

    in_maps = []
    for c in range(NCORE):
        b = c // GPB
        g0 = (c % GPB) * HL  # first global head of this core
        # q/k pair-blocks: [q(2g0..), q(..), k(..), k(..)] each 128 cols
        qcols = [Wq[:, (g0 + 2 * p) * 64:(g0 + 2 * p + 2) * 64] * SCALE
                 for p in range(HL // 2)]
        kcols = [Wk[:, (g0 + 2 * p) * 64:(g0 + 2 * p + 2) * 64]
                 for p in range(HL // 2)]
        wqk_c = np.ascontiguousarray(np.concatenate(qcols + kcols, axis=1))
        wv_c = np.ascontiguousarray(Wv[:, g0 * 64:(g0 + HL) * 64])
        bqk_c = np.stack(
            [bq[(g0 + 2 * p) * 64:(g0 + 2 * p + 2) * 64] * SCALE
             for p in range(HL // 2)]
            + [bk[(g0 + 2 * p) * 64:(g0 + 2 * p + 2) * 64]
               for p in range(HL // 2)], axis=1)
        bv_c = np.tile(bv_full[g0 * 64:(g0 + HL) * 64], (128, 1))
        wo_c = np.concatenate(
            [W_o[(g0 + h) * 64:(g0 + h + 1) * 64, :] for h in range(HL)],
            axis=1)
        in_maps.append({
            "xT": xT_b[b],
            "wqk": wqk_c.astype(np.float32),
            "wv": wv_c.astype(np.float32),
            "bqk": np.ascontiguousarray(bqk_c, dtype=np.float32),
            "bv": np.ascontiguousarray(bv_c, dtype=np.float32),
            "wo": np.ascontiguousarray(wo_c, dtype=np.float32),
            "maskT": maskT_b[b],
        })
    return in_maps


def combine_outputs(results, b_o):
    """results: list of 8 dicts with 'outp' [HL, Dh, S] and 'ssum' [HL, S]."""
    b_o = np.asarray(b_o, np.float32)
    out = np.zeros((B, S, Dh), np.float32)
    for c in range(NCORE):
        b = c // GPB
        op = results[c]["outp"].astype(np.float32)    # [HL, Dh, S]
        ss = results[c]["ssum"].astype(np.float32)    # [HL, S]
        contrib = (op / ss[:, None, :]).sum(axis=0)   # [Dh, S]
        out[b] += contrib.T
    out += b_o[None, None, :]
    return out


def kernel(x, W_qkv, b_qkv, W_o, b_o, mask):
    nc = get_module()
    in_maps = make_in_maps(x, W_qkv, b_qkv, W_o, b_o, mask)
    res = run_bass_kernel_spmd(nc, in_maps, core_ids=list(range(NCORE)))
    return combine_outputs(res.results, b_o)



# revision 25
# speedup vs baseline: 36.4378x; 36.4378x over previous
"""Multi-head attention Trainium2 Bass kernel (8 NeuronCores).

Problem: B=2, S=2048, D=1024, H=16, Dh=64, scale=1/sqrt(D).
Sharding: batch x head. Core c handles batch c//4, heads (c%4)*4 .. +4.
No collectives: per-core partial outputs are combined on the host
(softmax normalization + head sum + b_o add), which is O(B*H*S*Dh) work.

All matmul operands are bf16 (PSUM accumulation f32); weights and x
arrive from the host pre-cast to bf16 so no staging copies are needed.
SCALE is folded into Wq host-side.

Program order is arranged so the Scalar/Vector engines start early:
  K projection (all S) -> Q projection (first 512) -> logits+exp+mask
  for (qg=0, head pair 0) -> V projection -> Q rest -> ctx/proj for
  qg0 -> standard loop for qg 1..3.  Mask DMAs for qg0 are issued at
  kernel start.

Per-core pipeline:
  1. QKV projection from pre-transposed xT [D,S] in bf16:
       qT,kT per head-pair in [128, S] tiles (q rows 0-63 / 64-127 ...),
       v in natural [sk, d] layout with a fused ones column per head.
  2. Attention per (q-block of 512, head):
       logitsT [sk,sq] = k_chunk @ qT  (16 chunks of 128 sk)
       exp on ScalarE (PSUM -> bf16 SBUF), multiplicative {0,1} bf16
       mask on VectorE (2x mode), then ctxT'[65, 512] accumulated as
       (v|1).T @ P — row 64 = softmax denominators.
  3. Per-head output projection Wo_h.T @ ctx on device; DMA -> HBM.
"""

import numpy as np
import ml_dtypes

import concourse.bass as bass  # noqa: F401
import concourse.tile as tile
from concourse import bacc, mybir
from concourse.bass_utils import run_bass_kernel_spmd

B, S, D = 2, 2048, 1024
H, Dh = 16, 64
NCORE = 8
GPB = NCORE // B            # cores per batch (4)
HL = H // GPB               # local heads per core (4)
SCALE = float(1.0 / np.sqrt(np.float32(D)))

F32 = mybir.dt.float32
BF16 = mybir.dt.bfloat16
NPBF = ml_dtypes.bfloat16

PT_DT = BF16     # attention probability storage
V_DT = BF16      # v storage (must match PT_DT for the ctx matmul)

ND = D // 128    # 8 contraction chunks
NCH = S // 128   # 16 sk chunks
NQG = S // 512   # 4 query groups


def build_module(reps=1):
    nc = bacc.Bacc("TRN2", target_bir_lowering=False, debug=False,
                   num_devices=NCORE)

    xT = nc.dram_tensor("xT", [D, S], BF16, kind="ExternalInput").ap()
    # wqk cols per d-chunk: [q pair0 | q pair1 | k pair0 | k pair1] x 128
    wqk = nc.dram_tensor("wqk", [D, 4 * 128], BF16, kind="ExternalInput").ap()
    wv = nc.dram_tensor("wv", [D, HL * Dh], BF16, kind="ExternalInput").ap()
    bqk = nc.dram_tensor("bqk", [128, 4], F32, kind="ExternalInput").ap()
    bv = nc.dram_tensor("bv", [128, HL * Dh], F32, kind="ExternalInput").ap()
    wo = nc.dram_tensor("wo", [Dh, HL * Dh], BF16, kind="ExternalInput").ap()
    maskT = nc.dram_tensor("maskT", [S, S], BF16, kind="ExternalInput").ap()
    outp = nc.dram_tensor("outp", [reps * HL, Dh, S], BF16,
                          kind="ExternalOutput").ap()
    ssum = nc.dram_tensor("ssum", [reps * HL, S], F32,
                          kind="ExternalOutput").ap()

    with tile.TileContext(nc) as tc:
        with (
            tc.tile_pool(name="const", bufs=1) as constp,
            tc.tile_pool(name="qk", bufs=1) as qkp,
            tc.tile_pool(name="vpool", bufs=1) as vpoolp,
            tc.tile_pool(name="xtp", bufs=1, side="right") as xtp,
            tc.tile_pool(name="maskp", bufs=3, side="right") as maskp,
            tc.tile_pool(name="ptp", bufs=3, side="right") as ptp,
            tc.tile_pool(name="ptps", space="PSUM", bufs=3) as ptpsp,
            tc.tile_pool(name="ctxps", space="PSUM", bufs=2) as ctxpsp,
            tc.tile_pool(name="cup", bufs=3, side="right") as cup,
        ):
            # ---------------- persistent tiles ----------------
            wqk_sb = constp.tile([128, ND * 512], BF16)
            wv_sb = constp.tile([128, ND * 256], BF16)
            wo_sb = constp.tile([Dh, HL * Dh], BF16)
            bqk_sb = constp.tile([128, 4], F32)
            bv_sb = constp.tile([128, HL * Dh], F32)

            # qT / kT per head pair: rows 0-63 head 2g, rows 64-127 head 2g+1
            qt = [qkp.tile([128, S], BF16, name=f"qt{g}") for g in range(2)]
            kt = [qkp.tile([128, S], BF16, name=f"kt{g}") for g in range(2)]
            # v in [sk, d] layout: per sk-chunk j, per head h: 64 v + ones
            v_sb = vpoolp.tile([128, NCH * HL * 65], V_DT)
            nc.vector.memset(
                v_sb.rearrange("p (m c) -> p m c", c=65)[:, :, 64:65], 1.0)
            # x tiles: all four 512-column groups stay resident
            xts = [xtp.tile([128, ND * 512], BF16, name=f"xt{i}")
                   for i in range(NQG)]

            def dma_x(sb4):
                for d in range(ND):
                    nc.sync.dma_start(
                        xts[sb4][:, d * 512:(d + 1) * 512],
                        xT[d * 128:(d + 1) * 128,
                           sb4 * 512:(sb4 + 1) * 512])

            def emit_qk_block(sb4, blk):
                """blk 0,1 = q pairs; 2,3 = k pairs."""
                ps = ptpsp.tile([128, 1024], F32, tag="ptps", bufs=3)
                for d in range(ND):
                    nc.tensor.matmul(
                        ps[:, 0:512],
                        lhsT=wqk_sb[:, d * 512 + blk * 128:
                                    d * 512 + (blk + 1) * 128],
                        rhs=xts[sb4][:, d * 512:(d + 1) * 512],
                        start=(d == 0), stop=(d == ND - 1))
                tgt = qt[blk] if blk < 2 else kt[blk - 2]
                nc.vector.tensor_scalar_add(
                    tgt[:, sb4 * 512:(sb4 + 1) * 512], ps[:, 0:512],
                    bqk_sb[:, blk:blk + 1])

            def emit_v(sb4, jlo=0, jhi=4):
                for jj in range(jlo, jhi):
                    j = sb4 * 4 + jj
                    psv = ptpsp.tile([128, 1024], F32, tag="ptps", bufs=3)
                    for d in range(ND):
                        nc.tensor.matmul(
                            psv[:, 0:HL * Dh],
                            lhsT=xts[sb4][:, d * 512 + jj * 128:
                                          d * 512 + jj * 128 + 128],
                            rhs=wv_sb[:, d * 256:(d + 1) * 256],
                            start=(d == 0), stop=(d == ND - 1))
                    nc.vector.tensor_add(
                        v_sb[:, j * (HL * 65):(j + 1) * (HL * 65)]
                            .rearrange("p (h c) -> p h c", h=HL)[:, :, 0:64],
                        psv[:, 0:HL * Dh].rearrange("p (h c) -> p h c", h=HL),
                        bv_sb.rearrange("p (h c) -> p h c", h=HL))

            def dma_mask(qg):
                mts = []
                for r in range(2):
                    mt = maskp.tile([128, 8 * 512], BF16, name=f"mt{r}",
                                    tag="mask", bufs=2)
                    nc.sync.dma_start(
                        mt.rearrange("p (j c) -> p j c", j=8),
                        maskT.rearrange("(j p) q -> p j q", p=128)
                             [:, 8 * r:8 * r + 8,
                              qg * 512:(qg + 1) * 512])
                    mts.append(mt)
                return mts

            def emit_logits_round(qg, g, mts, r, pre=None, weave=None):
                """logits + exp + mask for one round of 8 sk-chunks.

                pre: optional callback(jj) emitted before chunk jj's
                matmuls (used to interleave K-projection blocks)."""
                ptt = ptp.tile([128, 8 * 2 * 512], PT_DT,
                               name=f"pt{g}_{r}", tag="pt", bufs=4)
                for jj in range(8):
                    if weave is not None:
                        weave(jj)
                    if pre is not None and jj in pre:
                        pre[jj]()
                    j = r * 8 + jj
                    pps = ptpsp.tile([128, 1024], F32, tag="ptps",
                                     bufs=3)
                    for hh in range(2):
                        nc.tensor.matmul(
                            pps[:, hh * 512:(hh + 1) * 512],
                            lhsT=kt[g][hh * 64:(hh + 1) * 64,
                                       j * 128:(j + 1) * 128],
                            rhs=qt[g][hh * 64:(hh + 1) * 64,
                                      qg * 512:(qg + 1) * 512],
                            start=True, stop=True)
                    nc.scalar.activation(
                        ptt[:, jj * 1024:(jj + 1) * 1024], pps,
                        mybir.ActivationFunctionType.Exp)
                ptv = ptt.rearrange("p (j e c) -> p j e c", j=8, e=2)
                mtv = mts[r].rearrange("p (j c) -> p j c", j=8)
                for e in range(2):
                    nc.vector.tensor_mul(
                        ptv[:, :, e, :], ptv[:, :, e, :], mtv)
                return ptt

            def emit_logits(qg, g, mts):
                return [emit_logits_round(qg, g, mts, r) for r in range(2)]

            def emit_ctx_proj(qg, g, pts, _rep):
                for hh in range(2):
                    ctx = emit_ctx_acc(g, hh, pts[0], None, 0, 8)
                    emit_ctx_acc(g, hh, None, pts[1], 8, NCH, ctx)
                    emit_ctx_out(qg, g, hh, ctx, _rep)

            def emit_ctx_acc(g, hh, pt0, pt1, j0, j1, ctx=None):
                h = 2 * g + hh
                if ctx is None:
                    ctx = ctxpsp.tile([65, 512], F32, tag="ctx", bufs=2)
                for j in range(j0, j1):
                    pt = pt0 if j < 8 else pt1
                    nc.tensor.matmul(
                        ctx,
                        lhsT=v_sb[:, j * (HL * 65) + h * 65:
                                  j * (HL * 65) + (h + 1) * 65],
                        rhs=pt[:, (j % 8) * 1024 + hh * 512:
                               (j % 8) * 1024 + (hh + 1) * 512],
                        start=(j == 0), stop=(j == NCH - 1))
                return ctx

            def emit_ctx_out(qg, g, hh, ctx, _rep):
                if True:
                    h = 2 * g + hh
                    cu = cup.tile([65, 512], BF16, tag="cu", bufs=3)
                    nc.vector.tensor_copy(cu, ctx)
                    den = cup.tile([1, 512], F32, tag="den", bufs=3)
                    nc.vector.tensor_copy(den, ctx[64:65, :])
                    nc.sync.dma_start(
                        ssum[_rep * HL + h:_rep * HL + h + 1,
                             qg * 512:(qg + 1) * 512], den)
                    po = ctxpsp.tile([64, 512], F32, tag="ctx")
                    nc.tensor.matmul(
                        po, lhsT=wo_sb[:, h * 64:(h + 1) * 64],
                        rhs=cu[0:64, :], start=True, stop=True)
                    po_sb = cup.tile([64, 512], BF16, tag="po", bufs=3)
                    nc.vector.tensor_copy(po_sb, po)
                    nc.sync.dma_start(
                        outp[_rep * HL + h][:, qg * 512:(qg + 1) * 512],
                        po_sb)

            # ---------------- schedule ----------------
            def ctx_head(qg, g, pts, hh, _rep):
                ctx = emit_ctx_acc(g, hh, pts[0], None, 0, 8)
                emit_ctx_acc(g, hh, None, pts[1], 8, NCH, ctx)
                emit_ctx_out(qg, g, hh, ctx, _rep)

            for _rep in range(reps):
                if _rep == 0:
                    # critical-path inputs first: x(sb4=0) + wqk,
                    # chunk-interleaved so the first K matmul starts early
                    for d in range(ND):
                        nc.sync.dma_start(
                            xts[0][:, d * 512:(d + 1) * 512],
                            xT[d * 128:(d + 1) * 128, 0:512])
                        nc.sync.dma_start(
                            wqk_sb[:, d * 512:(d + 1) * 512],
                            wqk[d * 128:(d + 1) * 128, :].rearrange(
                                "(o p) c -> p (o c)", o=1))
                    nc.sync.dma_start(bqk_sb, bqk)
                    nc.sync.dma_start(bv_sb, bv)
                    nc.sync.dma_start(
                        wv_sb.rearrange("p (d c) -> p d c", d=ND),
                        wv.rearrange("(d p) c -> p d c", p=128))
                    for sb4 in range(1, NQG):
                        dma_x(sb4)
                    nc.sync.dma_start(wo_sb, wo)
                emit_qk_block(0, 2)      # k pair 0, chunks 0-3
                emit_qk_block(0, 0)      # q pair 0, qg0 block
                mts0 = dma_mask(0)
                # qg0 logits with K/V blocks interleaved to keep PE busy
                p00_0 = emit_logits_round(
                    0, 0, mts0, 0,
                    pre={2: lambda: emit_v(0, 0, 2),
                         4: lambda: emit_qk_block(1, 2)})
                p00_1 = emit_logits_round(
                    0, 0, mts0, 1,
                    pre={0: lambda: emit_qk_block(2, 2),
                         2: lambda: emit_v(0, 2, 4),
                         4: lambda: emit_qk_block(3, 2)})
                emit_qk_block(0, 1)      # q pair 1, qg0 block
                p01_0 = emit_logits_round(
                    0, 1, mts0, 0,
                    pre={0: lambda: emit_qk_block(0, 3),
                         2: lambda: emit_v(1, 0, 2),
                         4: lambda: emit_qk_block(1, 3)})
                p01_1 = emit_logits_round(
                    0, 1, mts0, 1,
                    pre={0: lambda: emit_qk_block(2, 3),
                         2: lambda: emit_v(1, 2, 4),
                         4: lambda: emit_qk_block(3, 3)})
                emit_v(2)
                emit_v(3)
                ctx_head(0, 0, [p00_0, p00_1], 0, _rep)
                emit_qk_block(1, 0)
                ctx_head(0, 0, [p00_0, p00_1], 1, _rep)
                emit_qk_block(1, 1)
                # steady-state pipeline: per group, logits rounds paired
                # with the previous group's ctx heads
                fill = {
                    (1, 0, 0): lambda: emit_qk_block(2, 0),
                    (1, 0, 1): lambda: emit_qk_block(2, 1),
                    (1, 1, 0): lambda: emit_qk_block(3, 0),
                    (1, 1, 1): lambda: emit_qk_block(3, 1),
                }
                prev = (0, 1, [p01_0, p01_1])
                for qg in range(1, NQG):
                    mts_cur = dma_mask(qg)
                    for g in range(2):
                        last = (qg == NQG - 1 and g == 1)
                        pg, pts_p = prev[1], prev[2]
                        rs = []
                        for r in range(2):
                            # weave prev head hh=r's ctx chunks between
                            # this round's logits chunks (2 mm per chunk)
                            pctx = ctxpsp.tile([65, 512], F32, tag="ctx",
                                               bufs=2)
                            ph = 2 * pg + r

                            def weave(jj, pctx=pctx, ph=ph, pts_p=pts_p,
                                      r=r):
                                for j in (2 * jj, 2 * jj + 1):
                                    nc.tensor.matmul(
                                        pctx,
                                        lhsT=v_sb[:, j * (HL * 65) + ph * 65:
                                                  j * (HL * 65)
                                                  + (ph + 1) * 65],
                                        rhs=pts_p[j // 8]
                                            [:, (j % 8) * 1024 + r * 512:
                                             (j % 8) * 1024 + (r + 1) * 512],
                                        start=(j == 0), stop=(j == NCH - 1))

                            f = fill.get((qg, g, r))
                            rr = emit_logits_round(qg, g, mts_cur, r,
                                                   pre={4: f} if f else None,
                                                   weave=weave)
                            rs.append(rr)
                            emit_ctx_out(prev[0], pg, r, pctx, _rep)
                            if last:
                                rs0 = rs[0]
                                if r == 0:
                                    own0 = emit_ctx_acc(g, 0, rs0, None,
                                                        0, 8)
                                else:
                                    own1 = emit_ctx_acc(g, 1, rs0, None,
                                                        0, 8)
                        prev = (qg, g, rs)
                # drain: last group, second halves only
                qg, g = NQG - 1, 1
                emit_ctx_acc(g, 0, None, prev[2][1], 8, NCH, own0)
                emit_ctx_out(qg, g, 0, own0, _rep)
                emit_ctx_acc(g, 1, None, prev[2][1], 8, NCH, own1)
                emit_ctx_out(qg, g, 1, own1, _rep)

    nc.compile()
    return nc


_NC_CACHE = {}


def get_module(reps=1):
    if reps not in _NC_CACHE:
        _NC_CACHE[reps] = build_module(reps)
    return _NC_CACHE[reps]


def make_in_maps(x, W_qkv, b_qkv, W_o, b_o, mask):
    x = np.asarray(x, np.float32)
    W_qkv = np.asarray(W_qkv, np.float32)
    b_qkv = np.asarray(b_qkv, np.float32)
    W_o = np.asarray(W_o, np.float32)
    mask = np.asarray(mask)

    # reference layout: W_qkv[:, h*3*Dh + {0..Dh | Dh..2Dh | 2Dh..3Dh}] =
    # q|k|v of head h (qkv.reshape(B,S,H,3*Dh) then split on last axis)
    W3 = W_qkv.reshape(D, H, 3 * Dh)
    b3 = b_qkv.reshape(H, 3 * Dh)
    Wq = np.ascontiguousarray(W3[:, :, :Dh].reshape(D, H * Dh))
    Wk = np.ascontiguousarray(W3[:, :, Dh:2 * Dh].reshape(D, H * Dh))
    Wv = np.ascontiguousarray(W3[:, :, 2 * Dh:].reshape(D, H * Dh))
    bq = np.ascontiguousarray(b3[:, :Dh].reshape(H * Dh))
    bk = np.ascontiguousarray(b3[:, Dh:2 * Dh].reshape(H * Dh))
    bv_full = np.ascontiguousarray(b3[:, 2 * Dh:].reshape(H * Dh))

    xT_b = [np.ascontiguousarray(x[b].T).astype(NPBF) for b in range(B)]
    maskT_b = [np.ascontiguousarray(
        (mask[b, 0] != 0).T.astype(NPBF)) for b in range(B)]

    in_maps = []
    for c in range(NCORE):
        b = c // GPB
        g0 = (c % GPB) * HL  # first global head of this core
        # q/k pair-blocks: [q(2g0..), q(..), k(..), k(..)] each 128 cols
        qcols = [Wq[:, (g0 + 2 * p) * 64:(g0 + 2 * p + 2) * 64] * SCALE
                 for p in range(HL // 2)]
        kcols = [Wk[:, (g0 + 2 * p) * 64:(g0 + 2 * p + 2) * 64]
                 for p in range(HL // 2)]
        wqk_c = np.ascontiguousarray(np.concatenate(qcols + kcols, axis=1))
        wv_c = np.ascontiguousarray(Wv[:, g0 * 64:(g0 + HL) * 64])
        bqk_c = np.stack(
            [bq[(g0 + 2 * p) * 64:(g0 + 2 * p + 2) * 64] * SCALE
             for p in range(HL // 2)]
            + [bk[(g0 + 2 * p) * 64:(g0 + 2 * p + 2) * 64]
               for p in range(HL // 2)], axis=1)
        bv_c = np.tile(bv_full[g0 * 64:(g0 + HL) * 64], (128, 1))
        wo_c = np.concatenate(
            [W_o[(g0 + h) * 64:(g0 + h + 1) * 64, :] for h in range(HL)],
            axis=1)
        in_maps.append({
            "xT": xT_b[b],
            "wqk": wqk_c.astype(NPBF),
            "wv": wv_c.astype(NPBF),
            "bqk": np.ascontiguousarray(bqk_c, dtype=np.float32),
            "bv": np.ascontiguousarray(bv_c, dtype=np.float32),
            "wo": wo_c.astype(NPBF),
            "maskT": maskT_b[b],
        })
    return in_maps


def combine_outputs(results, b_o):
    """results: list of 8 dicts with 'outp' [HL, Dh, S] and 'ssum' [HL, S]."""
    b_o = np.asarray(b_o, np.float32)
    out = np.zeros((B, S, Dh), np.float32)
    for c in range(NCORE):
        b = c // GPB
        op = results[c]["outp"].astype(np.float32)    # [HL, Dh, S]
        ss = results[c]["ssum"].astype(np.float32)    # [HL, S]
        contrib = (op / ss[:, None, :]).sum(axis=0)   # [Dh, S]
        out[b] += contrib.T
    out += b_o[None, None, :]
    return out


def kernel(x, W_qkv, b_qkv, W_o, b_o, mask):
    nc = get_module()
    in_maps = make_in_maps(x, W_qkv, b_qkv, W_o, b_o, mask)
    res = run_bass_kernel_spmd(nc, in_maps, core_ids=list(range(NCORE)))
    return combine_outputs(res.results, b_o)


# revision 30
# speedup vs baseline: 37.1204x; 1.0187x over previous
"""Multi-head attention Trainium2 Bass kernel (8 NeuronCores).

Problem: B=2, S=2048, D=1024, H=16, Dh=64, scale=1/sqrt(D).
Sharding: batch x head. Core c handles batch c//4, heads (c%4)*4 .. +4.
No collectives: per-core partial outputs are combined on the host
(softmax normalization + head sum + b_o add), which is O(B*H*S*Dh) work.

All matmul operands are bf16 (PSUM accumulation f32); weights and x
arrive from the host pre-cast to bf16 so no staging copies are needed.
SCALE is folded into Wq host-side.

Program order is arranged so the Scalar/Vector engines start early:
  K projection (all S) -> Q projection (first 512) -> logits+exp+mask
  for (qg=0, head pair 0) -> V projection -> Q rest -> ctx/proj for
  qg0 -> standard loop for qg 1..3.  Mask DMAs for qg0 are issued at
  kernel start.

Per-core pipeline:
  1. QKV projection from pre-transposed xT [D,S] in bf16:
       qT,kT per head-pair in [128, S] tiles (q rows 0-63 / 64-127 ...),
       v in natural [sk, d] layout with a fused ones column per head.
  2. Attention per (q-block of 512, head):
       logitsT [sk,sq] = k_chunk @ qT  (16 chunks of 128 sk)
       exp on ScalarE (PSUM -> bf16 SBUF), multiplicative {0,1} bf16
       mask on VectorE (2x mode), then ctxT'[65, 512] accumulated as
       (v|1).T @ P — row 64 = softmax denominators.
  3. Per-head output projection Wo_h.T @ ctx on device; DMA -> HBM.
"""

import numpy as np
import ml_dtypes

import concourse.bass as bass  # noqa: F401
import concourse.tile as tile
from concourse import bacc, mybir
from concourse.bass_utils import run_bass_kernel_spmd

B, S, D = 2, 2048, 1024
H, Dh = 16, 64
NCORE = 8
GPB = NCORE // B            # cores per batch (4)
HL = H // GPB               # local heads per core (4)
SCALE = float(1.0 / np.sqrt(np.float32(D)))

F32 = mybir.dt.float32
BF16 = mybir.dt.bfloat16
NPBF = ml_dtypes.bfloat16

PT_DT = BF16     # attention probability storage
V_DT = BF16      # v storage (must match PT_DT for the ctx matmul)

ND = D // 128    # 8 contraction chunks
NCH = S // 128   # 16 sk chunks
NQG = S // 512   # 4 query groups


def build_module(reps=1):
    nc = bacc.Bacc("TRN2", target_bir_lowering=False, debug=False,
                   num_devices=NCORE)

    xT = nc.dram_tensor("xT", [D, S], BF16, kind="ExternalInput").ap()
    # wqk cols per d-chunk: [q pair0 | q pair1 | k pair0 | k pair1] x 128
    wqk = nc.dram_tensor("wqk", [D, 4 * 128], BF16, kind="ExternalInput").ap()
    wv = nc.dram_tensor("wv", [D, HL * Dh], BF16, kind="ExternalInput").ap()
    bqk = nc.dram_tensor("bqk", [128, 4], F32, kind="ExternalInput").ap()
    bv = nc.dram_tensor("bv", [128, HL * Dh], F32, kind="ExternalInput").ap()
    wo = nc.dram_tensor("wo", [Dh, HL * Dh], BF16, kind="ExternalInput").ap()
    maskT = nc.dram_tensor("maskT", [S, S], BF16, kind="ExternalInput").ap()
    outp = nc.dram_tensor("outp", [reps * HL, Dh, S], BF16,
                          kind="ExternalOutput").ap()
    ssum = nc.dram_tensor("ssum", [reps * HL, S], F32,
                          kind="ExternalOutput").ap()

    with tile.TileContext(nc) as tc:
        with (
            tc.tile_pool(name="const", bufs=1) as constp,
            tc.tile_pool(name="qk", bufs=1) as qkp,
            tc.tile_pool(name="vpool", bufs=1) as vpoolp,
            tc.tile_pool(name="xtp", bufs=1, side="right") as xtp,
            tc.tile_pool(name="maskp", bufs=3, side="right") as maskp,
            tc.tile_pool(name="ptp", bufs=3, side="right") as ptp,
            tc.tile_pool(name="ptps", space="PSUM", bufs=3) as ptpsp,
            tc.tile_pool(name="ctxps", space="PSUM", bufs=2) as ctxpsp,
            tc.tile_pool(name="cup", bufs=3, side="right") as cup,
        ):
            # ---------------- persistent tiles ----------------
            wqk_sb = constp.tile([128, ND * 512], BF16)
            wv_sb = constp.tile([128, ND * 256], BF16)
            wo_sb = constp.tile([Dh, HL * Dh], BF16)
            bqk_sb = constp.tile([128, 4], F32)
            bv_sb = constp.tile([128, HL * Dh], F32)

            # qT / kT per head pair: rows 0-63 head 2g, rows 64-127 head 2g+1
            qt = [qkp.tile([128, S], BF16, name=f"qt{g}") for g in range(2)]
            kt = [qkp.tile([128, S], BF16, name=f"kt{g}") for g in range(2)]
            # v in [sk, d] layout: per sk-chunk j, per head h: 64 v + ones
            v_sb = vpoolp.tile([128, NCH * HL * 65], V_DT)
            nc.vector.memset(
                v_sb.rearrange("p (m c) -> p m c", c=65)[:, :, 64:65], 1.0)
            # x tiles: all four 512-column groups stay resident
            xts = [xtp.tile([128, ND * 512], BF16, name=f"xt{i}")
                   for i in range(NQG)]

            def dma_x(sb4):
                for d in range(ND):
                    nc.sync.dma_start(
                        xts[sb4][:, d * 512:(d + 1) * 512],
                        xT[d * 128:(d + 1) * 128,
                           sb4 * 512:(sb4 + 1) * 512])

            def emit_qk_block(sb4, blk):
                """blk 0,1 = q pairs; 2,3 = k pairs."""
                ps = ptpsp.tile([128, 1024], F32, tag="ptps", bufs=3)
                for d in range(ND):
                    nc.tensor.matmul(
                        ps[:, 0:512],
                        lhsT=wqk_sb[:, d * 512 + blk * 128:
                                    d * 512 + (blk + 1) * 128],
                        rhs=xts[sb4][:, d * 512:(d + 1) * 512],
                        start=(d == 0), stop=(d == ND - 1))
                tgt = qt[blk] if blk < 2 else kt[blk - 2]
                nc.vector.tensor_scalar_add(
                    tgt[:, sb4 * 512:(sb4 + 1) * 512], ps[:, 0:512],
                    bqk_sb[:, blk:blk + 1])

            def emit_v(sb4, jlo=0, jhi=4):
                for jj in range(jlo, jhi):
                    j = sb4 * 4 + jj
                    psv = ptpsp.tile([128, 1024], F32, tag="ptps", bufs=3)
                    for d in range(ND):
                        nc.tensor.matmul(
                            psv[:, 0:HL * Dh],
                            lhsT=xts[sb4][:, d * 512 + jj * 128:
                                          d * 512 + jj * 128 + 128],
                            rhs=wv_sb[:, d * 256:(d + 1) * 256],
                            start=(d == 0), stop=(d == ND - 1))
                    nc.vector.tensor_add(
                        v_sb[:, j * (HL * 65):(j + 1) * (HL * 65)]
                            .rearrange("p (h c) -> p h c", h=HL)[:, :, 0:64],
                        psv[:, 0:HL * Dh].rearrange("p (h c) -> p h c", h=HL),
                        bv_sb.rearrange("p (h c) -> p h c", h=HL))

            def dma_mask(qg):
                mts = []
                for r in range(2):
                    mt = maskp.tile([128, 8 * 512], BF16, name=f"mt{r}",
                                    tag="mask", bufs=2)
                    nc.sync.dma_start(
                        mt.rearrange("p (j c) -> p j c", j=8),
                        maskT.rearrange("(j p) q -> p j q", p=128)
                             [:, 8 * r:8 * r + 8,
                              qg * 512:(qg + 1) * 512])
                    mts.append(mt)
                return mts

            def emit_logits_round(qg, g, mts, r, pre=None, weave=None,
                                  mask_split=False, ptt_out=None):
                """logits + exp + mask for one round of 8 sk-chunks.

                pre: optional callback(jj) emitted before chunk jj's
                matmuls (used to interleave K-projection blocks).
                mask_split: emit the mask multiply for chunks 0-3 right
                after their exps (lets consumers start mid-round)."""
                ptt = ptp.tile([128, 8 * 2 * 512], PT_DT,
                               name=f"pt{g}_{r}", tag="pt", bufs=4)
                if ptt_out is not None:
                    ptt_out.append(ptt)
                ptv = ptt.rearrange("p (j e c) -> p j e c", j=8, e=2)
                mtv = mts[r].rearrange("p (j c) -> p j c", j=8)
                for jj in range(8):
                    if weave is not None:
                        weave(jj)
                    if pre is not None and jj in pre:
                        pre[jj]()
                    j = r * 8 + jj
                    pps = ptpsp.tile([128, 1024], F32, tag="ptps",
                                     bufs=3)
                    for hh in range(2):
                        nc.tensor.matmul(
                            pps[:, hh * 512:(hh + 1) * 512],
                            lhsT=kt[g][hh * 64:(hh + 1) * 64,
                                       j * 128:(j + 1) * 128],
                            rhs=qt[g][hh * 64:(hh + 1) * 64,
                                      qg * 512:(qg + 1) * 512],
                            start=True, stop=True)
                    nc.scalar.activation(
                        ptt[:, jj * 1024:(jj + 1) * 1024], pps,
                        mybir.ActivationFunctionType.Exp)
                    if mask_split and jj in (3, 5):
                        sl = slice(0, 4) if jj == 3 else slice(4, 6)
                        for e in range(2):
                            nc.vector.tensor_mul(
                                ptv[:, sl, e, :], ptv[:, sl, e, :],
                                mtv[:, sl, :])
                jlo = 6 if mask_split else 0
                for e in range(2):
                    nc.vector.tensor_mul(
                        ptv[:, jlo:8, e, :], ptv[:, jlo:8, e, :],
                        mtv[:, jlo:8, :])
                return ptt

            def emit_logits(qg, g, mts):
                return [emit_logits_round(qg, g, mts, r) for r in range(2)]

            def emit_ctx_proj(qg, g, pts, _rep):
                for hh in range(2):
                    ctx = emit_ctx_acc(g, hh, pts[0], None, 0, 8)
                    emit_ctx_acc(g, hh, None, pts[1], 8, NCH, ctx)
                    emit_ctx_out(qg, g, hh, ctx, _rep)

            def emit_ctx_acc(g, hh, pt0, pt1, j0, j1, ctx=None):
                h = 2 * g + hh
                if ctx is None:
                    ctx = ctxpsp.tile([65, 512], F32, tag="ctx", bufs=2)
                for j in range(j0, j1):
                    pt = pt0 if j < 8 else pt1
                    nc.tensor.matmul(
                        ctx,
                        lhsT=v_sb[:, j * (HL * 65) + h * 65:
                                  j * (HL * 65) + (h + 1) * 65],
                        rhs=pt[:, (j % 8) * 1024 + hh * 512:
                               (j % 8) * 1024 + (hh + 1) * 512],
                        start=(j == 0), stop=(j == NCH - 1))
                return ctx

            def emit_ctx_out(qg, g, hh, ctx, _rep):
                if True:
                    h = 2 * g + hh
                    cu = cup.tile([65, 512], BF16, tag="cu", bufs=3)
                    nc.vector.tensor_copy(cu, ctx)
                    po = ctxpsp.tile([64, 512], F32, tag="ctx")
                    nc.tensor.matmul(
                        po, lhsT=wo_sb[:, h * 64:(h + 1) * 64],
                        rhs=cu[0:64, :], start=True, stop=True)
                    den = cup.tile([1, 512], F32, tag="den", bufs=3)
                    nc.vector.tensor_copy(den, ctx[64:65, :])
                    nc.sync.dma_start(
                        ssum[_rep * HL + h:_rep * HL + h + 1,
                             qg * 512:(qg + 1) * 512], den)
                    po_sb = cup.tile([64, 512], BF16, tag="po", bufs=3)
                    nc.vector.tensor_copy(po_sb, po)
                    nc.sync.dma_start(
                        outp[_rep * HL + h][:, qg * 512:(qg + 1) * 512],
                        po_sb)

            # ---------------- schedule ----------------
            def ctx_head(qg, g, pts, hh, _rep):
                ctx = emit_ctx_acc(g, hh, pts[0], None, 0, 8)
                emit_ctx_acc(g, hh, None, pts[1], 8, NCH, ctx)
                emit_ctx_out(qg, g, hh, ctx, _rep)

            for _rep in range(reps):
                if _rep == 0:
                    # critical-path inputs first: x(sb4=0) + wqk,
                    # chunk-interleaved so the first K matmul starts early
                    for d in range(ND):
                        nc.sync.dma_start(
                            xts[0][:, d * 512:(d + 1) * 512],
                            xT[d * 128:(d + 1) * 128, 0:512])
                        nc.sync.dma_start(
                            wqk_sb[:, d * 512:(d + 1) * 512],
                            wqk[d * 128:(d + 1) * 128, :].rearrange(
                                "(o p) c -> p (o c)", o=1))
                    nc.sync.dma_start(bqk_sb, bqk)
                    nc.sync.dma_start(bv_sb, bv)
                    nc.sync.dma_start(
                        wv_sb.rearrange("p (d c) -> p d c", d=ND),
                        wv.rearrange("(d p) c -> p d c", p=128))
                    for sb4 in range(1, NQG):
                        dma_x(sb4)
                    nc.sync.dma_start(wo_sb, wo)
                emit_qk_block(0, 2)      # k pair 0, chunks 0-3
                emit_qk_block(0, 0)      # q pair 0, qg0 block
                mts0 = dma_mask(0)
                # qg0 logits with K/V blocks interleaved to keep PE busy
                p00_0 = emit_logits_round(
                    0, 0, mts0, 0,
                    pre={4: lambda: emit_qk_block(1, 2),
                         6: lambda: emit_v(0, 0, 2)})
                p00_1 = emit_logits_round(
                    0, 0, mts0, 1,
                    pre={0: lambda: emit_qk_block(2, 2),
                         4: lambda: emit_qk_block(3, 2),
                         6: lambda: emit_v(0, 2, 4)})
                emit_qk_block(0, 1)      # q pair 1, qg0 block
                p01_0 = emit_logits_round(
                    0, 1, mts0, 0,
                    pre={0: lambda: emit_qk_block(0, 3),
                         4: lambda: emit_qk_block(1, 3),
                         6: lambda: emit_v(1, 0, 2)})
                p01_1 = emit_logits_round(
                    0, 1, mts0, 1,
                    pre={0: lambda: emit_qk_block(2, 3),
                         4: lambda: emit_qk_block(3, 3),
                         6: lambda: emit_v(1, 2, 4)})
                emit_v(2)
                emit_v(3)
                ctx_head(0, 0, [p00_0, p00_1], 0, _rep)
                emit_qk_block(1, 0)
                ctx_head(0, 0, [p00_0, p00_1], 1, _rep)
                emit_qk_block(1, 1)
                # steady-state pipeline: per group, logits rounds paired
                # with the previous group's ctx heads
                fill = {
                    (1, 0, 0): lambda: emit_qk_block(2, 0),
                    (1, 0, 1): lambda: emit_qk_block(2, 1),
                    (1, 1, 0): lambda: emit_qk_block(3, 0),
                    (1, 1, 1): lambda: emit_qk_block(3, 1),
                }
                prev = (0, 1, [p01_0, p01_1])
                for qg in range(1, NQG):
                    mts_cur = dma_mask(qg)
                    for g in range(2):
                        last = (qg == NQG - 1 and g == 1)
                        pg, pts_p = prev[1], prev[2]
                        rs = []
                        owns = {}
                        for r in range(2):
                            lastr = last and r == 1
                            cell = []
                            if lastr:
                                # prev head h1's ctx as a plain block,
                                # then this group's own first halves, so
                                # the ctx ring holds only the own tiles
                                # during the final round
                                ctx_head(prev[0], pg, pts_p, 1, _rep)
                                owns[0] = emit_ctx_acc(g, 0, rs[0], None,
                                                       0, 8)
                                owns[1] = emit_ctx_acc(g, 1, rs[0], None,
                                                       0, 8)

                                def weave(jj):
                                    if jj in (4, 5, 6):
                                        for hh in range(2):
                                            j0 = 8 + 2 * (jj - 4)
                                            emit_ctx_acc(1, hh, None,
                                                         cell[0], j0,
                                                         j0 + 2, owns[hh])
                            else:
                                pctx = ctxpsp.tile([65, 512], F32,
                                                   tag="ctx", bufs=2)
                                ph = 2 * pg + r

                                def weave(jj, pctx=pctx, ph=ph,
                                          pts_p=pts_p, r=r):
                                    for j in (2 * jj, 2 * jj + 1):
                                        nc.tensor.matmul(
                                            pctx,
                                            lhsT=v_sb[:, j * (HL * 65)
                                                      + ph * 65:
                                                      j * (HL * 65)
                                                      + (ph + 1) * 65],
                                            rhs=pts_p[j // 8]
                                                [:, (j % 8) * 1024 + r * 512:
                                                 (j % 8) * 1024
                                                 + (r + 1) * 512],
                                            start=(j == 0),
                                            stop=(j == NCH - 1))

                            f = fill.get((qg, g, r))
                            rr = emit_logits_round(qg, g, mts_cur, r,
                                                   pre={4: f} if f else None,
                                                   weave=weave,
                                                   mask_split=lastr,
                                                   ptt_out=cell)
                            rs.append(rr)
                            if not lastr:
                                emit_ctx_out(prev[0], pg, r, pctx, _rep)
                        prev = (qg, g, rs)
                # drain: last group, final two chunks only
                qg, g = NQG - 1, 1
                emit_ctx_acc(g, 0, None, prev[2][1], 14, NCH, owns[0])
                emit_ctx_out(qg, g, 0, owns[0], _rep)
                emit_ctx_acc(g, 1, None, prev[2][1], 14, NCH, owns[1])
                emit_ctx_out(qg, g, 1, owns[1], _rep)

    nc.compile()
    return nc


_NC_CACHE = {}


def get_module(reps=1):
    if reps not in _NC_CACHE:
        _NC_CACHE[reps] = build_module(reps)
    return _NC_CACHE[reps]


def make_in_maps(x, W_qkv, b_qkv, W_o, b_o, mask):
    x = np.asarray(x, np.float32)
    W_qkv = np.asarray(W_qkv, np.float32)
    b_qkv = np.asarray(b_qkv, np.float32)
    W_o = np.asarray(W_o, np.float32)
    mask = np.asarray(mask)

    # reference layout: W_qkv[:, h*3*Dh + {0..Dh | Dh..2Dh | 2Dh..3Dh}] =
    # q|k|v of head h (qkv.reshape(B,S,H,3*Dh) then split on last axis)
    W3 = W_qkv.reshape(D, H, 3 * Dh)
    b3 = b_qkv.reshape(H, 3 * Dh)
    Wq = np.ascontiguousarray(W3[:, :, :Dh].reshape(D, H * Dh))
    Wk = np.ascontiguousarray(W3[:, :, Dh:2 * Dh].reshape(D, H * Dh))
    Wv = np.ascontiguousarray(W3[:, :, 2 * Dh:].reshape(D, H * Dh))
    bq = np.ascontiguousarray(b3[:, :Dh].reshape(H * Dh))
    bk = np.ascontiguousarray(b3[:, Dh:2 * Dh].reshape(H * Dh))
    bv_full = np.ascontiguousarray(b3[:, 2 * Dh:].reshape(H * Dh))

    xT_b = [np.ascontiguousarray(x[b].T).astype(NPBF) for b in range(B)]
    maskT_b = [np.ascontiguousarray(
        (mask[b, 0] != 0).T.astype(NPBF)) for b in range(B)]

    in_maps = []
    for c in range(NCORE):
        b = c // GPB
        g0 = (c % GPB) * HL  # first global head of this core
        # q/k pair-blocks: [q(2g0..), q(..), k(..), k(..)] each 128 cols
        qcols = [Wq[:, (g0 + 2 * p) * 64:(g0 + 2 * p + 2) * 64] * SCALE
                 for p in range(HL // 2)]
        kcols = [Wk[:, (g0 + 2 * p) * 64:(g0 + 2 * p + 2) * 64]
                 for p in range(HL // 2)]
        wqk_c = np.ascontiguousarray(np.concatenate(qcols + kcols, axis=1))
        wv_c = np.ascontiguousarray(Wv[:, g0 * 64:(g0 + HL) * 64])
        bqk_c = np.stack(
            [bq[(g0 + 2 * p) * 64:(g0 + 2 * p + 2) * 64] * SCALE
             for p in range(HL // 2)]
            + [bk[(g0 + 2 * p) * 64:(g0 + 2 * p + 2) * 64]
               for p in range(HL // 2)], axis=1)
        bv_c = np.tile(bv_full[g0 * 64:(g0 + HL) * 64], (128, 1))
        wo_c = np.concatenate(
            [W_o[(g0 + h) * 64:(g0 + h + 1) * 64, :] for h in range(HL)],
            axis=1)
        in_maps.append({
            "xT": xT_b[b],
            "wqk": wqk_c.astype(NPBF),
            "wv": wv_c.astype(NPBF),
            "bqk": np.ascontiguousarray(bqk_c, dtype=np.float32),
            "bv": np.ascontiguousarray(bv_c, dtype=np.float32),
            "wo": wo_c.astype(NPBF),
            "maskT": maskT_b[b],
        })
    return in_maps


def combine_outputs(results, b_o):
    """results: list of 8 dicts with 'outp' [HL, Dh, S] and 'ssum' [HL, S]."""
    b_o = np.asarray(b_o, np.float32)
    out = np.zeros((B, S, Dh), np.float32)
    for c in range(NCORE):
        b = c // GPB
        op = results[c]["outp"].astype(np.float32)    # [HL, Dh, S]
        ss = results[c]["ssum"].astype(np.float32)    # [HL, S]
        contrib = (op / ss[:, None, :]).sum(axis=0)   # [Dh, S]
        out[b] += contrib.T
    out += b_o[None, None, :]
    return out


def kernel(x, W_qkv, b_qkv, W_o, b_o, mask):
    nc = get_module()
    in_maps = make_in_maps(x, W_qkv, b_qkv, W_o, b_o, mask)
    res = run_bass_kernel_spmd(nc, in_maps, core_ids=list(range(NCORE)))
    return combine_outputs(res.results, b_o)


# revision 37
# speedup vs baseline: 38.4645x; 1.0362x over previous
"""Multi-head attention Trainium2 Bass kernel (8 NeuronCores).

Problem: B=2, S=2048, D=1024, H=16, Dh=64, scale=1/sqrt(D).
Sharding: batch x head. Core c handles batch c//4, heads (c%4)*4 .. +4.
No collectives: per-core partial outputs are combined on the host
(softmax normalization + head sum + b_o add), which is O(B*H*S*Dh) work.

All matmul operands are bf16 (PSUM accumulation f32); weights and x
arrive from the host pre-cast to bf16 so no staging copies are needed.
SCALE is folded into Wq host-side.

Program order is arranged so the Scalar/Vector engines start early:
  K projection (all S) -> Q projection (first 512) -> logits+exp+mask
  for (qg=0, head pair 0) -> V projection -> Q rest -> ctx/proj for
  qg0 -> standard loop for qg 1..3.  Mask DMAs for qg0 are issued at
  kernel start.

Per-core pipeline:
  1. QKV projection from pre-transposed xT [D,S] in bf16:
       qT,kT per head-pair in [128, S] tiles (q rows 0-63 / 64-127 ...),
       v in natural [sk, d] layout with a fused ones column per head.
  2. Attention per (q-block of 512, head):
       logitsT [sk,sq] = k_chunk @ qT  (16 chunks of 128 sk)
       exp on ScalarE (PSUM -> bf16 SBUF), multiplicative {0,1} bf16
       mask on VectorE (2x mode), then ctxT'[65, 512] accumulated as
       (v|1).T @ P — row 64 = softmax denominators.
  3. Per-head output projection Wo_h.T @ ctx on device; DMA -> HBM.
"""

import numpy as np
import ml_dtypes

import concourse.bass as bass  # noqa: F401
import concourse.tile as tile
from concourse import bacc, mybir
from concourse.bass_utils import run_bass_kernel_spmd

B, S, D = 2, 2048, 1024
H, Dh = 16, 64
NCORE = 8
GPB = NCORE // B            # cores per batch (4)
HL = H // GPB               # local heads per core (4)
SCALE = float(1.0 / np.sqrt(np.float32(D)))

F32 = mybir.dt.float32
BF16 = mybir.dt.bfloat16
NPBF = ml_dtypes.bfloat16

PT_DT = BF16     # attention probability storage
V_DT = BF16      # v storage (must match PT_DT for the ctx matmul)

ND = D // 128    # 8 contraction chunks
NCH = S // 128   # 16 sk chunks
NQG = S // 512   # 4 query groups


def build_module(reps=1):
    nc = bacc.Bacc("TRN2", target_bir_lowering=False, debug=False,
                   num_devices=NCORE)

    xT = nc.dram_tensor("xT", [D, S], BF16, kind="ExternalInput").ap()
    # wqk cols per d-chunk: [q pair0 | q pair1 | k pair0 | k pair1] x 128
    wqk = nc.dram_tensor("wqk", [D, 4 * 128], BF16, kind="ExternalInput").ap()
    wv = nc.dram_tensor("wv", [D, HL * Dh], BF16, kind="ExternalInput").ap()
    bqk = nc.dram_tensor("bqk", [128, 4], F32, kind="ExternalInput").ap()
    bv = nc.dram_tensor("bv", [128, HL * Dh], F32, kind="ExternalInput").ap()
    wo = nc.dram_tensor("wo", [Dh, HL * Dh], BF16, kind="ExternalInput").ap()
    maskT = nc.dram_tensor("maskT", [S, S], BF16, kind="ExternalInput").ap()
    outp = nc.dram_tensor("outp", [reps * HL, Dh, S], BF16,
                          kind="ExternalOutput").ap()
    ssum = nc.dram_tensor("ssum", [reps * HL, S], F32,
                          kind="ExternalOutput").ap()

    with tile.TileContext(nc) as tc:
        with (
            tc.tile_pool(name="const", bufs=1) as constp,
            tc.tile_pool(name="qk", bufs=1) as qkp,
            tc.tile_pool(name="vpool", bufs=1) as vpoolp,
            tc.tile_pool(name="xtp", bufs=1, side="right") as xtp,
            tc.tile_pool(name="maskp", bufs=3, side="right") as maskp,
            tc.tile_pool(name="ptp", bufs=3, side="right") as ptp,
            tc.tile_pool(name="ptps", space="PSUM", bufs=3) as ptpsp,
            tc.tile_pool(name="ctxps", space="PSUM", bufs=2) as ctxpsp,
            tc.tile_pool(name="cup", bufs=3, side="right") as cup,
        ):
            # ---------------- persistent tiles ----------------
            wqk_sb = constp.tile([128, ND * 512], BF16)
            wv_sb = constp.tile([128, ND * 256], BF16)
            wo_sb = constp.tile([Dh, HL * Dh], BF16)
            bqk_sb = constp.tile([128, 4], F32)
            bv_sb = constp.tile([128, HL * Dh], F32)

            # qT / kT per head pair: rows 0-63 head 2g, rows 64-127 head 2g+1
            qt = [qkp.tile([128, S], BF16, name=f"qt{g}") for g in range(2)]
            kt = [qkp.tile([128, S], BF16, name=f"kt{g}") for g in range(2)]
            # v in [sk, d] layout: per sk-chunk j, per head h: 64 v + ones
            v_sb = vpoolp.tile([128, NCH * HL * 65], V_DT)
            nc.vector.memset(
                v_sb.rearrange("p (m c) -> p m c", c=65)[:, :, 64:65], 1.0)
            # x tiles: all four 512-column groups stay resident
            xts = [xtp.tile([128, ND * 512], BF16, name=f"xt{i}")
                   for i in range(NQG)]

            def dma_x(sb4):
                for d in range(ND):
                    nc.sync.dma_start(
                        xts[sb4][:, d * 512:(d + 1) * 512],
                        xT[d * 128:(d + 1) * 128,
                           sb4 * 512:(sb4 + 1) * 512])

            def emit_qk_block(sb4, blk, c0=0, c1=512):
                """blk 0,1 = q pairs; 2,3 = k pairs. [c0,c1) = s-columns
                within the 512-block (smaller first slice lets the first
                logits chunk start sooner)."""
                ps = ptpsp.tile([128, 1024], F32, tag="ptps", bufs=3)
                for d in range(ND):
                    nc.tensor.matmul(
                        ps[:, 0:c1 - c0],
                        lhsT=wqk_sb[:, d * 512 + blk * 128:
                                    d * 512 + (blk + 1) * 128],
                        rhs=xts[sb4][:, d * 512 + c0:d * 512 + c1],
                        start=(d == 0), stop=(d == ND - 1))
                tgt = qt[blk] if blk < 2 else kt[blk - 2]
                nc.vector.tensor_scalar_add(
                    tgt[:, sb4 * 512 + c0:sb4 * 512 + c1],
                    ps[:, 0:c1 - c0], bqk_sb[:, blk:blk + 1])

            def emit_v(sb4, jlo=0, jhi=4):
                for jj in range(jlo, jhi):
                    j = sb4 * 4 + jj
                    psv = ptpsp.tile([128, 1024], F32, tag="ptps", bufs=3)
                    for d in range(ND):
                        nc.tensor.matmul(
                            psv[:, 0:HL * Dh],
                            lhsT=xts[sb4][:, d * 512 + jj * 128:
                                          d * 512 + jj * 128 + 128],
                            rhs=wv_sb[:, d * 256:(d + 1) * 256],
                            start=(d == 0), stop=(d == ND - 1))
                    nc.vector.tensor_add(
                        v_sb[:, j * (HL * 65):(j + 1) * (HL * 65)]
                            .rearrange("p (h c) -> p h c", h=HL)[:, :, 0:64],
                        psv[:, 0:HL * Dh].rearrange("p (h c) -> p h c", h=HL),
                        bv_sb.rearrange("p (h c) -> p h c", h=HL))

            def dma_mask(qg):
                mts = []
                for r in range(2):
                    mt = maskp.tile([128, 8 * 512], BF16, name=f"mt{r}",
                                    tag="mask", bufs=2)
                    nc.sync.dma_start(
                        mt.rearrange("p (j c) -> p j c", j=8),
                        maskT.rearrange("(j p) q -> p j q", p=128)
                             [:, 8 * r:8 * r + 8,
                              qg * 512:(qg + 1) * 512])
                    mts.append(mt)
                return mts

            def emit_logits_round(qg, g, mts, r, pre=None, weave=None,
                                  mask_split=False, ptt_out=None):
                """logits + exp + mask for one round of 8 sk-chunks.

                pre: optional callback(jj) emitted before chunk jj's
                matmuls (used to interleave K-projection blocks).
                mask_split: emit the mask multiply for chunks 0-3 right
                after their exps (lets consumers start mid-round)."""
                ptt = ptp.tile([128, 8 * 2 * 512], PT_DT,
                               name=f"pt{g}_{r}", tag="pt", bufs=4)
                if ptt_out is not None:
                    ptt_out.append(ptt)
                ptv = ptt.rearrange("p (j e c) -> p j e c", j=8, e=2)
                mtv = mts[r].rearrange("p (j c) -> p j c", j=8)
                for jj in range(8):
                    if weave is not None:
                        weave(jj)
                    if pre is not None and jj in pre:
                        pre[jj]()
                    j = r * 8 + jj
                    pps = ptpsp.tile([128, 1024], F32, tag="ptps",
                                     bufs=3)
                    for hh in range(2):
                        nc.tensor.matmul(
                            pps[:, hh * 512:(hh + 1) * 512],
                            lhsT=kt[g][hh * 64:(hh + 1) * 64,
                                       j * 128:(j + 1) * 128],
                            rhs=qt[g][hh * 64:(hh + 1) * 64,
                                      qg * 512:(qg + 1) * 512],
                            start=True, stop=True)
                    nc.scalar.activation(
                        ptt[:, jj * 1024:(jj + 1) * 1024], pps,
                        mybir.ActivationFunctionType.Exp)
                    if mask_split and jj in (3, 5):
                        sl = slice(0, 4) if jj == 3 else slice(4, 6)
                        for e in range(2):
                            nc.vector.tensor_mul(
                                ptv[:, sl, e, :], ptv[:, sl, e, :],
                                mtv[:, sl, :])
                jlo = 6 if mask_split else 0
                for e in range(2):
                    nc.vector.tensor_mul(
                        ptv[:, jlo:8, e, :], ptv[:, jlo:8, e, :],
                        mtv[:, jlo:8, :])
                return ptt

            def emit_logits(qg, g, mts):
                return [emit_logits_round(qg, g, mts, r) for r in range(2)]

            def emit_ctx_proj(qg, g, pts, _rep):
                for hh in range(2):
                    ctx = emit_ctx_acc(g, hh, pts[0], None, 0, 8)
                    emit_ctx_acc(g, hh, None, pts[1], 8, NCH, ctx)
                    emit_ctx_out(qg, g, hh, ctx, _rep)

            def emit_ctx_acc(g, hh, pt0, pt1, j0, j1, ctx=None):
                h = 2 * g + hh
                if ctx is None:
                    ctx = ctxpsp.tile([65, 512], F32, tag="ctx", bufs=2)
                for j in range(j0, j1):
                    pt = pt0 if j < 8 else pt1
                    nc.tensor.matmul(
                        ctx,
                        lhsT=v_sb[:, j * (HL * 65) + h * 65:
                                  j * (HL * 65) + (h + 1) * 65],
                        rhs=pt[:, (j % 8) * 1024 + hh * 512:
                               (j % 8) * 1024 + (hh + 1) * 512],
                        start=(j == 0), stop=(j == NCH - 1))
                return ctx

            def emit_ctx_out(qg, g, hh, ctx, _rep):
                if True:
                    h = 2 * g + hh
                    cu = cup.tile([65, 512], BF16, tag="cu", bufs=3)
                    nc.vector.tensor_copy(cu, ctx)
                    po = ctxpsp.tile([64, 512], F32, tag="ctx")
                    nc.tensor.matmul(
                        po, lhsT=wo_sb[:, h * 64:(h + 1) * 64],
                        rhs=cu[0:64, :], start=True, stop=True)
                    den = cup.tile([1, 512], F32, tag="den", bufs=3)
                    nc.vector.tensor_copy(den, ctx[64:65, :])
                    # alternate output DMA queues (SP / Pool) so the two
                    # final head chains drain in parallel
                    eng = nc.sync if hh == 0 else nc.gpsimd
                    eng.dma_start(
                        ssum[_rep * HL + h:_rep * HL + h + 1,
                             qg * 512:(qg + 1) * 512], den)
                    po_sb = cup.tile([64, 512], BF16, tag="po", bufs=3)
                    nc.vector.tensor_copy(po_sb, po)
                    eng.dma_start(
                        outp[_rep * HL + h][:, qg * 512:(qg + 1) * 512],
                        po_sb)

            # ---------------- schedule ----------------
            def ctx_head(qg, g, pts, hh, _rep):
                ctx = emit_ctx_acc(g, hh, pts[0], None, 0, 8)
                emit_ctx_acc(g, hh, None, pts[1], 8, NCH, ctx)
                emit_ctx_out(qg, g, hh, ctx, _rep)

            warm = vpoolp.tile([128, 512], BF16, name="warm")
            for _rep in range(reps):
                if _rep == 0:
                    # critical-path inputs first: x(sb4=0) + wqk, spread
                    # across the SP/Pool/Act DMA queues so the chunks
                    # land in parallel
                    for d in range(ND):
                        eng = nc.sync if d % 2 == 0 else nc.scalar
                        eng.dma_start(
                            xts[0][:, d * 512:(d + 1) * 512],
                            xT[d * 128:(d + 1) * 128, 0:512])
                        nc.gpsimd.dma_start(
                            wqk_sb[:, d * 512:(d + 1) * 512],
                            wqk[d * 128:(d + 1) * 128, :].rearrange(
                                "(o p) c -> p (o c)", o=1))
                    nc.gpsimd.dma_start(bqk_sb, bqk)
                    nc.gpsimd.dma_start(bv_sb, bv)
                    nc.gpsimd.dma_start(
                        wv_sb.rearrange("p (d c) -> p d c", d=ND),
                        wv.rearrange("(d p) c -> p d c", p=128))
                    for sb4 in range(1, NQG):
                        for d in range(ND):
                            eng = nc.sync if d % 2 == 0 else nc.gpsimd
                            eng.dma_start(
                                xts[sb4][:, d * 512:(d + 1) * 512],
                                xT[d * 128:(d + 1) * 128,
                                   sb4 * 512:(sb4 + 1) * 512])
                    nc.gpsimd.dma_start(wo_sb, wo)
                if _rep == 0:
                    # PE pstate warmup on scratch data while input DMAs
                    # land (results discarded)
                    nc.vector.memset(warm, 0.0)
                    for _ in range(5):
                        wps = ptpsp.tile([128, 1024], F32, tag="ptps",
                                         bufs=3)
                        nc.tensor.matmul(
                            wps[:, 0:512], lhsT=warm[:, 0:128],
                            rhs=warm, start=True, stop=True)
                emit_qk_block(0, 2, 0, 128)   # k pair 0, chunk 0 only
                emit_qk_block(0, 0)      # q pair 0, qg0 block
                mts0 = dma_mask(0)
                # qg0 logits with K/V blocks interleaved to keep PE busy
                p00_0 = emit_logits_round(
                    0, 0, mts0, 0,
                    pre={1: lambda: emit_qk_block(0, 2, 128, 512),
                         4: lambda: emit_qk_block(1, 2),
                         6: lambda: emit_v(0, 0, 2)})
                p00_1 = emit_logits_round(
                    0, 0, mts0, 1,
                    pre={0: lambda: emit_qk_block(2, 2),
                         4: lambda: emit_qk_block(3, 2),
                         6: lambda: emit_v(0, 2, 4)})
                emit_v(2, 0, 2)
                emit_qk_block(0, 1)      # q pair 1, qg0 block
                p01_0 = emit_logits_round(
                    0, 1, mts0, 0,
                    pre={0: lambda: emit_qk_block(0, 3),
                         4: lambda: emit_qk_block(1, 3),
                         6: lambda: emit_v(1, 0, 2)})
                p01_1 = emit_logits_round(
                    0, 1, mts0, 1,
                    pre={0: lambda: emit_qk_block(2, 3),
                         4: lambda: emit_qk_block(3, 3),
                         6: lambda: emit_v(1, 2, 4)})
                emit_v(2, 2, 4)
                emit_v(3)
                ctx_head(0, 0, [p00_0, p00_1], 0, _rep)
                emit_qk_block(1, 0)
                ctx_head(0, 0, [p00_0, p00_1], 1, _rep)
                emit_qk_block(1, 1)
                # steady-state pipeline: per group, logits rounds paired
                # with the previous group's ctx heads
                fill = {
                    (1, 0, 0): lambda: emit_qk_block(2, 0),
                    (1, 0, 1): lambda: emit_qk_block(2, 1),
                    (1, 1, 0): lambda: emit_qk_block(3, 0),
                    (1, 1, 1): lambda: emit_qk_block(3, 1),
                }
                prev = (0, 1, [p01_0, p01_1])
                for qg in range(1, NQG):
                    mts_cur = dma_mask(qg)
                    for g in range(2):
                        last = (qg == NQG - 1 and g == 1)
                        pg, pts_p = prev[1], prev[2]
                        rs = []
                        owns = {}
                        for r in range(2):
                            lastr = last and r == 1
                            cell = []
                            if lastr:
                                # prev head h1's ctx as a plain block,
                                # then this group's own first halves, so
                                # the ctx ring holds only the own tiles
                                # during the final round
                                ctx_head(prev[0], pg, pts_p, 1, _rep)
                                owns[0] = emit_ctx_acc(g, 0, rs[0], None,
                                                       0, 8)
                                owns[1] = emit_ctx_acc(g, 1, rs[0], None,
                                                       0, 8)

                                def weave(jj):
                                    if jj in (4, 5, 6):
                                        for hh in range(2):
                                            j0 = 8 + 2 * (jj - 4)
                                            emit_ctx_acc(1, hh, None,
                                                         cell[0], j0,
                                                         j0 + 2, owns[hh])
                            else:
                                pctx = ctxpsp.tile([65, 512], F32,
                                                   tag="ctx", bufs=2)
                                ph = 2 * pg + r

                                def weave(jj, pctx=pctx, ph=ph,
                                          pts_p=pts_p, r=r):
                                    for j in (2 * jj, 2 * jj + 1):
                                        nc.tensor.matmul(
                                            pctx,
                                            lhsT=v_sb[:, j * (HL * 65)
                                                      + ph * 65:
                                                      j * (HL * 65)
                                                      + (ph + 1) * 65],
                                            rhs=pts_p[j // 8]
                                                [:, (j % 8) * 1024 + r * 512:
                                                 (j % 8) * 1024
                                                 + (r + 1) * 512],
                                            start=(j == 0),
                                            stop=(j == NCH - 1))

                            f = fill.get((qg, g, r))
                            rr = emit_logits_round(qg, g, mts_cur, r,
                                                   pre={4: f} if f else None,
                                                   weave=weave,
                                                   mask_split=lastr,
                                                   ptt_out=cell)
                            rs.append(rr)
                            if not lastr:
                                emit_ctx_out(prev[0], pg, r, pctx, _rep)
                        prev = (qg, g, rs)
                # drain: last group, final two chunks only
                qg, g = NQG - 1, 1
                emit_ctx_acc(g, 0, None, prev[2][1], 14, NCH, owns[0])
                emit_ctx_out(qg, g, 0, owns[0], _rep)
                emit_ctx_acc(g, 1, None, prev[2][1], 14, NCH, owns[1])
                emit_ctx_out(qg, g, 1, owns[1], _rep)

    nc.compile()
    return nc


_NC_CACHE = {}


def get_module(reps=1):
    if reps not in _NC_CACHE:
        _NC_CACHE[reps] = build_module(reps)
    return _NC_CACHE[reps]


def make_in_maps(x, W_qkv, b_qkv, W_o, b_o, mask):
    x = np.asarray(x, np.float32)
    W_qkv = np.asarray(W_qkv, np.float32)
    b_qkv = np.asarray(b_qkv, np.float32)
    W_o = np.asarray(W_o, np.float32)
    mask = np.asarray(mask)

    # reference layout: W_qkv[:, h*3*Dh + {0..Dh | Dh..2Dh | 2Dh..3Dh}] =
    # q|k|v of head h (qkv.reshape(B,S,H,3*Dh) then split on last axis)
    W3 = W_qkv.reshape(D, H, 3 * Dh)
    b3 = b_qkv.reshape(H, 3 * Dh)
    Wq = np.ascontiguousarray(W3[:, :, :Dh].reshape(D, H * Dh))
    Wk = np.ascontiguousarray(W3[:, :, Dh:2 * Dh].reshape(D, H * Dh))
    Wv = np.ascontiguousarray(W3[:, :, 2 * Dh:].reshape(D, H * Dh))
    bq = np.ascontiguousarray(b3[:, :Dh].reshape(H * Dh))
    bk = np.ascontiguousarray(b3[:, Dh:2 * Dh].reshape(H * Dh))
    bv_full = np.ascontiguousarray(b3[:, 2 * Dh:].reshape(H * Dh))

    xT_b = [np.ascontiguousarray(x[b].T).astype(NPBF) for b in range(B)]
    maskT_b = [np.ascontiguousarray(
        (mask[b, 0] != 0).T.astype(NPBF)) for b in range(B)]

    in_maps = []
    for c in range(NCORE):
        b = c // GPB
        g0 = (c % GPB) * HL  # first global head of this core
        # q/k pair-blocks: [q(2g0..), q(..), k(..), k(..)] each 128 cols
        qcols = [Wq[:, (g0 + 2 * p) * 64:(g0 + 2 * p + 2) * 64] * SCALE
                 for p in range(HL // 2)]
        kcols = [Wk[:, (g0 + 2 * p) * 64:(g0 + 2 * p + 2) * 64]
                 for p in range(HL // 2)]
        wqk_c = np.ascontiguousarray(np.concatenate(qcols + kcols, axis=1))
        wv_c = np.ascontiguousarray(Wv[:, g0 * 64:(g0 + HL) * 64])
        bqk_c = np.stack(
            [bq[(g0 + 2 * p) * 64:(g0 + 2 * p + 2) * 64] * SCALE
             for p in range(HL // 2)]
            + [bk[(g0 + 2 * p) * 64:(g0 + 2 * p + 2) * 64]
               for p in range(HL // 2)], axis=1)
        bv_c = np.tile(bv_full[g0 * 64:(g0 + HL) * 64], (128, 1))
        wo_c = np.concatenate(
            [W_o[(g0 + h) * 64:(g0 + h + 1) * 64, :] for h in range(HL)],
            axis=1)
        in_maps.append({
            "xT": xT_b[b],
            "wqk": wqk_c.astype(NPBF),
            "wv": wv_c.astype(NPBF),
            "bqk": np.ascontiguousarray(bqk_c, dtype=np.float32),
            "bv": np.ascontiguousarray(bv_c, dtype=np.float32),
            "wo": wo_c.astype(NPBF),
            "maskT": maskT_b[b],
        })
    return in_maps


def combine_outputs(results, b_o):
    """results: list of 8 dicts with 'outp' [HL, Dh, S] and 'ssum' [HL, S]."""
    b_o = np.asarray(b_o, np.float32)
    out = np.zeros((B, S, Dh), np.float32)
    for c in range(NCORE):
        b = c // GPB
        op = results[c]["outp"].astype(np.float32)    # [HL, Dh, S]
        ss = results[c]["ssum"].astype(np.float32)    # [HL, S]
        contrib = (op / ss[:, None, :]).sum(axis=0)   # [Dh, S]
        out[b] += contrib.T
    out += b_o[None, None, :]
    return out


def kernel(x, W_qkv, b_qkv, W_o, b_o, mask):
    nc = get_module()
    in_maps = make_in_maps(x, W_qkv, b_qkv, W_o, b_o, mask)
    res = run_bass_kernel_spmd(nc, in_maps, core_ids=list(range(NCORE)))
    return combine_outputs(res.results, b_o)
